# revision 28
# baseline (speedup 1.0000x reference)
"""Trainium2 Bass kernel for: ConvTranspose2d(128->256, k=4, s=2, p=1)
-> MaxPool2d(2,2) -> Hardtanh -> spatial mean -> Tanh.

Polyphase decomposition (as the bf16 baseline): the stride-2 transposed conv
splits into 4 polyphase 2x2 convolutions whose outputs at pooled position
(i, j) are exactly the 4 elements of the 2x2 maxpool window, so everything
stays at 64x64 and the 128x128 conv output is never materialized.

This version gets ~2x more PE throughput from fp8(e4m3) matmuls in DoubleRow
perf mode: the PE virtualizes to 128x256 (two fp8 weights per cell), so the
two ROW taps of each phase become a single matmul with contraction 256.  The
moving operand is a 3D AP [cin=128, pair=2, free] over a zero-padded 66x66
canvas where the pair dim strides one canvas row (+66) and the column tap is
a flat +-1 offset; the free dim covers whole 66-wide rows (7 rows = 462 <=
512 fp32 = one PSUM bank) including 2 ignorable pad columns per row.

Weights are pre-scaled by S=64 before the fp8 cast (w std ~0.022 would land
in e4m3's subnormal range); the scale and the conv bias are folded into
per-channel Hardtanh clip bounds lo = S*(-1-b), hi = S*(1-b) and into the
final Tanh's scale/bias, so no separate bias-add pass exists.

Downstream per chunk (4 phase banks in PSUM), measured-optimal split:
ScalarE is the ONLY efficient PSUM reader (DVE/PSUM ops stall against the
PE's PSUM writes; DMA cannot address PSUM), so ACT evacuates all 4 banks
to bf16 SBUF in one ACTIVATE(Copy) per chunk, then on DVE:
  m     = TT-max of plane pairs        [max(p0,p2) | max(p1,p3)]
  s2    = (m_a max lo) max m_b         one fused scalar_tensor_tensor
  sum  += min(s2, hi)                  one tensor_scalar with add-accum
Final: out = Tanh(sum/(4096*S) + b) on ACT.  Steady state is ACT-bound at
~1.8us/chunk vs PE ~1.4us/chunk (~284us/core vs bf16 baseline ~511us).

Sharding: data-parallel over batch, 8 images per core on 8 cores.
"""

from contextlib import ExitStack

import ml_dtypes
import numpy as np

import concourse.bacc as bacc
import concourse.bass as bass
import concourse.mybir as mybir
import concourse.tile as tile
from concourse.bass_utils import run_bass_kernel_spmd

# Problem dims (hardcoded per contract)
B, CIN, COUT, H, W = 64, 128, 256, 64, 64
NCORES = 8
BPC = B // NCORES  # images per core

WP = 66  # padded row width (1 + 64 + 1)
NROW = 66  # padded rows (1 + 64 + 1)
CVT = WP * NROW + 2  # lead/tail guard bytes for the dj=+-1 flat offsets

# Output rows 1..64 of the canvas grid: 9 chunks of 7 rows + 1 tail row.
CHUNKS = [(1 + 7 * i, 7) for i in range(9)] + [(64, 1)]
NCH = len(CHUNKS)

WSCALE = 64.0  # weight pre-scale before fp8 cast

F32 = mybir.dt.float32
BF16 = mybir.dt.bfloat16
FP8 = mybir.dt.float8e4

# Per (img, half): chunk indices whose phase-max reads banks 2:4 from PSUM
# on DVE (1x mode) with only a half ACT evacuation (chained; slow - unused).
DIRECT_CHUNKS = ()
# Chunk indices where DVE independently copies banks 0:2 out of PSUM while
# ACT evacuates banks 2:4 - splits the PSUM-read stream across both engines
# (ACT alone is (FD+352)/1.2ns per op and would be the bottleneck at ~265us).
DVE_EVAC_CHUNKS = ()


def _tap(ph: int, a: int):
    """For phase parity ph (0=even output coord, 1=odd) and tap index a,
    return (input shift, kernel index) in one dimension.

    ConvTranspose2d(stride=2, pad=1): out[2q+r] = sum over taps of
    x[q+di] * w[k].  r=0: (di,k) in {(0,1), (-1,3)}; r=1: {(1,0), (0,2)}.
    """
    if ph == 0:
        return (0, 1) if a == 0 else (-1, 3)
    return (1, 0) if a == 0 else (0, 2)


def build_nc(
    n_imgs: int = BPC,
    repeat: int = 1,
    direct_chunks=DIRECT_CHUNKS,
    dve_evac_chunks=DVE_EVAC_CHUNKS,
    dve_sep_chunks=(),
    gpsimd_s3: bool = False,
    evac_full: bool = False,
    deep_bufs: bool = True,
    perf_mode=None,
    swi: bool = True,
    split_evac: bool = False,
    skip_downstream: bool = False,
) -> bass.Bass:
    """repeat>1 wraps the whole compute in a hardware loop executing it
    `repeat` times - used only for wall-clock timing (amortizes the ~80ms
    axon RPC overhead); the graded path uses repeat=1 (no loop)."""
    if perf_mode is None:
        perf_mode = (
            mybir.MatmulPerfMode.DoubleRowSwInterleave
            if swi
            else mybir.MatmulPerfMode.DoubleRow
        )
    nc = bacc.Bacc("TRN2", target_bir_lowering=False, debug=False)

    xc = nc.dram_tensor("xc", [BPC, 128, CVT], FP8, kind="ExternalInput")
    wm = nc.dram_tensor("wm", [128, 16 * 2 * 128], FP8, kind="ExternalInput")
    br = nc.dram_tensor("br", [128, 2], F32, kind="ExternalInput")
    clo = nc.dram_tensor("clo", [128, 2], F32, kind="ExternalInput")
    chi = nc.dram_tensor("chi", [128, 2], F32, kind="ExternalInput")
    out = nc.dram_tensor("out", [128, 2 * BPC], F32, kind="ExternalOutput")

    Copy = mybir.ActivationFunctionType.Copy
    Tanh = mybir.ActivationFunctionType.Tanh
    MAX = mybir.AluOpType.max
    MIN = mybir.AluOpType.min
    ADD = mybir.AluOpType.add

    with ExitStack() as ctx:
        tc = ctx.enter_context(tile.TileContext(nc))
        consts = ctx.enter_context(tc.tile_pool(name="consts", bufs=1))
        canvp = ctx.enter_context(
            tc.tile_pool(name="canv", bufs=4 if deep_bufs else 3)
        )
        psump = ctx.enter_context(tc.tile_pool(name="ps", bufs=2, space="PSUM"))
        evacp = ctx.enter_context(
            tc.tile_pool(name="ev", bufs=4 if deep_bufs else 3)
        )
        nb = 5 if deep_bufs else 3
        mpool = ctx.enter_context(tc.tile_pool(name="mt", bufs=nb))
        s2pool = ctx.enter_context(tc.tile_pool(name="s2", bufs=nb))
        cpool = ctx.enter_context(tc.tile_pool(name="ct", bufs=nb))
        accp = ctx.enter_context(tc.tile_pool(name="acc", bufs=3))

        w_sb = consts.tile([128, 16, 2, 128], FP8, tag="w")
        nc.sync.dma_start(
            w_sb[:].rearrange("p a b c -> p (a b c)"), wm[:, :]
        )
        b_sb = consts.tile([128, 2], F32, tag="b")
        nc.sync.dma_start(b_sb[:], br[:, :])
        lo_sb = consts.tile([128, 2], F32, tag="lo")
        nc.sync.dma_start(lo_sb[:], clo[:, :])
        hi_sb = consts.tile([128, 2], F32, tag="hi")
        nc.sync.dma_start(hi_sb[:], chi[:, :])
        s_all = consts.tile([128, 2 * BPC], F32, tag="sums")
        nc.vector.memset(s_all[:], 0.0)
        o_sb = consts.tile([128, 2 * BPC], F32, tag="out")
        nc.vector.memset(o_sb[:], 0.0)

        def body():
            for img in range(n_imgs):
                canv = canvp.tile([128, CVT], FP8, tag="canv")
                nc.sync.dma_start(canv[:], xc[img])
                for half in range(2):
                    acc = accp.tile([128, NCH], F32, tag="acc")
                    for ci, (r0, nr) in enumerate(CHUNKS):
                        nf = nr * WP
                        nv = nr * 64
                        ps = psump.tile([128, 4, 512], F32, tag="ps")
                        for p in range(4):
                            ph, pw = p >> 1, p & 1
                            di0 = _tap(ph, 0)[0]
                            rowbase = r0 + di0 - 1
                            for bb in range(2):
                                dj = _tap(pw, bb)[0]
                                base = 1 + rowbase * WP + dj
                                v = canv[:, base : base + nf].unsqueeze(1)
                                v.ap[1] = [WP, 2]
                                wv = w_sb[:, (half * 4 + p) * 2 + bb]
                                if swi:
                                    wv = wv.rearrange("p a b -> p (a b)")
                                nc.tensor.matmul(
                                    ps[:, p, :nf],
                                    wv,
                                    v,
                                    start=(bb == 0),
                                    stop=(bb == 1),
                                    perf_mode=perf_mode,
                                )
                        if skip_downstream:
                            continue
                        # phase-max pair stage.  HW allows only one PSUM
                        # input per DVE op, so the "direct" variant ACT-evacs
                        # banks 0:2 (ready early, overlaps phase-2/3 matmuls)
                        # and maxes them against banks 2:4 read from PSUM.
                        m = mpool.tile([128, 2, 7, 64], BF16, tag="m")
                        if ci in direct_chunks and nr == 7:
                            ev = evacp.tile([128, 4, 7, 64], BF16, tag="ev")
                            pva = ps[:, 0:2, 1:65].unsqueeze(2)
                            pva.ap[2] = [WP, nr]
                            nc.scalar.activation(ev[:, 0:2, :nr, :], pva, Copy)
                            pvb = ps[:, 2:4, 1:65].unsqueeze(2)
                            pvb.ap[2] = [WP, nr]
                            nc.vector.tensor_tensor(
                                m[:, :, :nr, :], pvb, ev[:, 0:2, :nr, :], MAX
                            )
                        elif ci in dve_sep_chunks and nr == 7:
                            # decoupled tiles: DVE copies banks 0:2 into its
                            # own tile while ACT evacs banks 2:4 into another
                            evd = evacp.tile([128, 2, 7, 64], BF16, tag="evd")
                            pva = ps[:, 0:2, 1:65].unsqueeze(2)
                            pva.ap[2] = [WP, nr]
                            nc.vector.tensor_copy(evd[:, :, :nr, :], pva)
                            evh = evacp.tile([128, 2, 7, 64], BF16, tag="evh")
                            pvb = ps[:, 2:4, 1:65].unsqueeze(2)
                            pvb.ap[2] = [WP, nr]
                            nc.scalar.activation(evh[:, :, :nr, :], pvb, Copy)
                            nc.vector.tensor_tensor(
                                m[:, :, :nr, :],
                                evd[:, :, :nr, :],
                                evh[:, :, :nr, :],
                                MAX,
                            )
                        elif ci in dve_evac_chunks and nr == 7:
                            ev = evacp.tile([128, 4, 7, 64], BF16, tag="ev")
                            pva = ps[:, 0:2, 1:65].unsqueeze(2)
                            pva.ap[2] = [WP, nr]
                            nc.vector.tensor_copy(ev[:, 0:2, :nr, :], pva)
                            pvb = ps[:, 2:4, 1:65].unsqueeze(2)
                            pvb.ap[2] = [WP, nr]
                            nc.scalar.activation(ev[:, 2:4, :nr, :], pvb, Copy)
                            nc.vector.tensor_tensor(
                                m[:, :, :nr, :],
                                ev[:, 0:2, :nr, :],
                                ev[:, 2:4, :nr, :],
                                MAX,
                            )
                        elif evac_full:
                            # contiguous evac incl. the 2 pad cols per row;
                            # downstream APs skip them
                            evf = evacp.tile([128, 4, 462], BF16, tag="evfu")
                            nc.scalar.activation(
                                evf[:, :, :nf], ps[:, 0:4, :nf], Copy
                            )
                            e4 = evf[:, :, 1:65].unsqueeze(2)
                            e4.ap[2] = [WP, nr]
                            nc.vector.tensor_tensor(
                                m[:, :, :nr, :], e4[:, 0:2], e4[:, 2:4], MAX
                            )
                        else:
                            ev = evacp.tile([128, 4, 7, 64], BF16, tag="ev")
                            if split_evac:
                                for h2 in range(2):
                                    pv = ps[:, 2 * h2 : 2 * h2 + 2, 1:65].unsqueeze(2)
                                    pv.ap[2] = [WP, nr]
                                    nc.scalar.activation(
                                        ev[:, 2 * h2 : 2 * h2 + 2, :nr, :], pv, Copy
                                    )
                            else:
                                pv = ps[:, 0:4, 1:65].unsqueeze(2)
                                pv.ap[2] = [WP, nr]
                                nc.scalar.activation(ev[:, :, :nr, :], pv, Copy)
                            nc.vector.tensor_tensor(
                                m[:, :, :nr, :],
                                ev[:, 0:2, :nr, :],
                                ev[:, 2:4, :nr, :],
                                MAX,
                            )
                        # (max . lower-clip) then (upper-clip . sum-accum)
                        s2 = s2pool.tile([128, 7, 64], BF16, tag="s2")
                        nc.vector.scalar_tensor_tensor(
                            s2[:, :nr, :],
                            m[:, 0, :nr, :],
                            lo_sb[:, half : half + 1],
                            m[:, 1, :nr, :],
                            MAX,
                            MAX,
                        )
                        cs = cpool.tile([128, 7, 64], BF16, tag="c")
                        s3_eng = (
                            nc.gpsimd
                            if (gpsimd_s3 and ci in dve_sep_chunks)
                            else nc.vector
                        )
                        s3_eng.tensor_scalar(
                            out=cs[:, :nr, :],
                            in0=s2[:, :nr, :],
                            scalar1=hi_sb[:, half : half + 1],
                            scalar2=None,
                            op0=MIN,
                            op1=ADD,
                            accum_out=acc[:, ci : ci + 1],
                        )
                    if not skip_downstream:
                        idx = half * n_imgs + img
                        nc.vector.reduce_sum(
                            s_all[:, idx : idx + 1],
                            acc[:, :NCH],
                            axis=mybir.AxisListType.X,
                        )

        if repeat > 1:
            with tc.For_i(0, repeat, 1):
                body()
        else:
            body()

        for half in range(2):
            sl = slice(half * n_imgs, (half + 1) * n_imgs)
            nc.scalar.activation(
                o_sb[:, sl],
                s_all[:, sl],
                Tanh,
                bias=b_sb[:, half : half + 1],
                scale=1.0 / (4096.0 * WSCALE),
            )
        nc.sync.dma_start(out[:, :], o_sb[:])

    nc.finalize()
    return nc


_CACHE: dict = {}


def _get_nc() -> bass.Bass:
    if "nc" not in _CACHE:
        _CACHE["nc"] = build_nc()
    return _CACHE["nc"]


def make_in_maps(x: np.ndarray, weight: np.ndarray, bias: np.ndarray,
                 swi: bool = True):
    x = np.asarray(x, dtype=np.float32)
    weight = np.asarray(weight, dtype=np.float32)
    bias = np.asarray(bias, dtype=np.float32)

    canv = np.zeros((B, 128, CVT), dtype=ml_dtypes.float8_e4m3)
    view = canv[:, :, 1 : 1 + WP * NROW].reshape(B, 128, NROW, WP)
    view[:, :, 1:65, 1:65] = x  # cast fp32 -> fp8

    # weight pairs: pair i=0 is the a=1 row tap (one canvas row up),
    # i=1 the a=0 row tap, matching the moving AP's +WP pair stride.
    # swi: DoubleRowSwInterleave expects [W0[127], W1[127], W0[126], ...]
    wmv = np.zeros((128, 16 * 2 * 128), dtype=ml_dtypes.float8_e4m3)
    for half in range(2):
        for p in range(4):
            ph, pw = p >> 1, p & 1
            for bb in range(2):
                kw = _tap(pw, bb)[1]
                base = (((half * 4 + p) * 2 + bb) * 2) * 128
                blk = wmv[:, base : base + 256]
                for i in range(2):
                    kh = _tap(ph, 1 - i)[1]
                    wq = (
                        WSCALE * weight[:, half * 128 : (half + 1) * 128, kh, kw]
                    ).astype(ml_dtypes.float8_e4m3)
                    if swi:
                        blk[:, i::2] = wq[:, ::-1]
                    else:
                        blk[:, i * 128 : (i + 1) * 128] = wq

    brv = np.ascontiguousarray(bias.reshape(2, 128).T, dtype=np.float32)
    clov = np.ascontiguousarray(WSCALE * (-1.0 - brv), dtype=np.float32)
    chiv = np.ascontiguousarray(WSCALE * (1.0 - brv), dtype=np.float32)

    return [
        {
            "xc": canv[c * BPC : (c + 1) * BPC],
            "wm": wmv,
            "br": brv,
            "clo": clov,
            "chi": chiv,
        }
        for c in range(NCORES)
    ]


def assemble_output(results: list) -> np.ndarray:
    outs = []
    for c in range(NCORES):
        o = np.asarray(results[c]["out"])  # [128, 2*BPC] = [part, half, img]
        o = o.reshape(128, 2, BPC).transpose(2, 1, 0).reshape(BPC, COUT)
        outs.append(o)
    return np.concatenate(outs, 0).reshape(B, COUT, 1, 1).astype(np.float32)


def kernel(x: np.ndarray, weight: np.ndarray, bias: np.ndarray) -> np.ndarray:
    nc = _get_nc()
    in_maps = make_in_maps(x, weight, bias)
    res = run_bass_kernel_spmd(nc, in_maps, core_ids=list(range(NCORES)))
    return assemble_output(res.results)
